# revision 1
# baseline (speedup 1.0000x reference)
"""Trainium2 Bass kernel for nn_AtteMatchLay (multi-perspective cosine matching).

Math (per flattened row n, perspective p):
    dot[n,p] = sum_d r[n,d]*m[n,d]*w2[p,d]
    n1s[n,p] = sum_d r[n,d]^2 * w2[p,d]        (w2 = weight**2)
    n2s[n,p] = sum_d m[n,d]^2 * w2[p,d]
    cos[n,p] = dot / (sqrt(n1s)*sqrt(n2s))

Strategy: data-parallel over the flattened N=16*512=8192 rows across 8 cores
(1024 rows each). The host hands each core its shard TRANSPOSED ([D, rows])
so the contraction dim D sits on SBUF partitions and the three contractions
run directly on the TensorEngine against the replicated [D, 20] weight —
no on-chip transposes. The elementwise products are split across the
Vector/Scalar/GpSimd engines; the epilogue (sqrt + reciprocal + multiply)
runs on-chip; the tiny [20, 1024] per-core result is re-transposed on host.
"""

import sys

if "/opt/trn_rl_repo" not in sys.path:
    sys.path.insert(0, "/opt/trn_rl_repo")

import numpy as np

# ---- problem constants (hardcoded per contract) ----
BSZ, SL, D, MP = 16, 512, 768, 20
N = BSZ * SL           # 8192 flattened rows
NCORES = 8
NSH = N // NCORES      # 1024 rows per core
P = 128                # SBUF partitions
NB = D // P            # 6 d-blocks
# row-groups per core: each group gets its own PSUM accumulators + epilogue.
GROUPS = [(0, 512), (512, 512)]  # (start, size); sizes must be <=512 (fp32 mm free dim)

_CACHE = {}


def _build():
    import concourse.tile as tile
    from concourse import bacc, mybir

    f32 = mybir.dt.float32
    bf16 = mybir.dt.bfloat16
    nc = bacc.Bacc(None, target_bir_lowering=False)

    rT = nc.dram_tensor("rT", [D, NSH], bf16, kind="ExternalInput")
    mT = nc.dram_tensor("mT", [D, NSH], bf16, kind="ExternalInput")
    w2T = nc.dram_tensor("w2T", [D, MP], bf16, kind="ExternalInput")
    out = nc.dram_tensor("out", [MP, NSH], f32, kind="ExternalOutput")

    SQ = mybir.ActivationFunctionType.Square
    ARSQRT = mybir.ActivationFunctionType.Abs_reciprocal_sqrt
    MUL = mybir.AluOpType.mult

    with tile.TileContext(nc) as tc:
        with (
            tc.tile_pool(name="const", bufs=1) as const,
            tc.tile_pool(name="inp", bufs=1) as inp,
            tc.tile_pool(name="prod", bufs=3) as prod,
            tc.tile_pool(name="epi", bufs=2) as epi,
            tc.tile_pool(name="psum", bufs=1, space="PSUM") as psum,
        ):
            w2_sb = const.tile([P, NB, MP], bf16, tag="w2")
            nc.sync.dma_start(
                out=w2_sb[:], in_=w2T.rearrange("(b p) q -> p b q", p=P)
            )

            r_sb = inp.tile([P, NB, NSH], bf16, tag="r")
            m_sb = inp.tile([P, NB, NSH], bf16, tag="m")

            # DMA loads: coalesced chunks (2 d-blocks early, 1 late) to cut
            # HWDGE issue serialization while keeping a fine-grained tail;
            # the final d-block arrives in group-halves so the last matmuls
            # and the epilogue start as soon as each half lands.
            for b0, nb in [(0, 2), (2, 2), (4, 1), (5, 1)]:
                for src, dst in ((rT, r_sb), (mT, m_sb)):
                    nc.sync.dma_start(
                        out=dst[:, b0 : b0 + nb, :],
                        in_=src[b0 * P : (b0 + nb) * P, :].rearrange(
                            "(c p) n -> p c n", p=P
                        ),
                    )

            dot_ps, n1_ps, n2_ps = [], [], []
            for gi, (gs, gn) in enumerate(GROUPS):
                dps = psum.tile([MP, gn], f32, tag=f"dot{gi}")
                n1p = psum.tile([MP, gn], f32, tag=f"n1{gi}")
                n2p = psum.tile([MP, gn], f32, tag=f"n2{gi}")
                dot_ps.append(dps)
                n1_ps.append(n1p)
                n2_ps.append(n2p)

            # Full-width bf16 products per d-block; matmuls slice per group.
            # Engine split balances measured busy time: DVE also carries the
            # epilogue, GpSimd carries 4 mm products, ACT carries rr.
            # The final d-block's products run per group-half so each group's
            # last matmul fires as soon as its half of the data lands.
            for b in range(NB):
                rsl = r_sb[:, b, :]
                msl = m_sb[:, b, :]
                rm = prod.tile([P, NSH], bf16, tag="rm")
                rr = prod.tile([P, NSH], bf16, tag="rr")
                mm = prod.tile([P, NSH], bf16, tag="mm")
                # DVE runs a pure rm chain (so rm5 is never queued behind an
                # mm); GpSimd takes only the first three mm (its ~2.6us/op
                # serial chain otherwise straggles past the DMA window and
                # stalls the PE's b3-b5 rounds); ACT absorbs mm3-5 at
                # ~1us/op interleaved with its input-gated rr ops.
                nc.vector.tensor_tensor(rm[:], rsl, msl, MUL)
                nc.scalar.activation(rr[:], rsl, SQ)
                if b < 3:
                    nc.gpsimd.tensor_tensor(mm[:], msl, msl, MUL)
                else:
                    nc.scalar.activation(mm[:], msl, SQ)

                w2b = w2_sb[:, b, :]
                st, sp = b == 0, b == NB - 1
                for gi, (gs, gn) in enumerate(GROUPS):
                    gsl = slice(gs, gs + gn)
                    nc.tensor.matmul(dot_ps[gi][:], w2b, rm[:, gsl], start=st, stop=sp)
                    nc.tensor.matmul(n1_ps[gi][:], w2b, rr[:, gsl], start=st, stop=sp)
                    nc.tensor.matmul(n2_ps[gi][:], w2b, mm[:, gsl], start=st, stop=sp)

            for gi, (gs, gn) in enumerate(GROUPS):
                # epilogue: cos = dot * rsqrt(n1s) * rsqrt(n2s); the
                # Abs_reciprocal_sqrt spline reads PSUM directly on ACT
                # (n1s/n2s > 0 so |x| is exact) and drops the separate
                # reciprocal op from the critical chain.
                u1 = epi.tile([MP, gn], f32, tag="u1")
                nc.scalar.activation(u1[:], n1_ps[gi][:], ARSQRT)
                u2 = epi.tile([MP, gn], f32, tag="u2")
                nc.scalar.activation(u2[:], n2_ps[gi][:], ARSQRT)
                t = epi.tile([MP, gn], f32, tag="t")
                nc.vector.tensor_tensor(t[:], u1[:], u2[:], MUL)
                cos = epi.tile([MP, gn], f32, tag="cos")
                nc.vector.tensor_tensor(cos[:], dot_ps[gi][:], t[:], MUL)
                nc.sync.dma_start(out=out[:, gs : gs + gn], in_=cos[:])

    nc.finalize()
    return nc


def get_nc():
    if "nc" not in _CACHE:
        _CACHE["nc"] = _build()
    return _CACHE["nc"]


def make_in_maps(repres, max_att, weight):
    import ml_dtypes

    r = np.ascontiguousarray(repres, dtype=np.float32).reshape(N, D)
    m = np.ascontiguousarray(max_att, dtype=np.float32).reshape(N, D)
    w2t = np.ascontiguousarray(
        (weight.astype(np.float32) ** 2).T.astype(ml_dtypes.bfloat16)
    )  # [D, MP] bf16
    in_maps = []
    for c in range(NCORES):
        rows = slice(c * NSH, (c + 1) * NSH)
        in_maps.append(
            {
                "rT": np.ascontiguousarray(r[rows].T.astype(ml_dtypes.bfloat16)),
                "mT": np.ascontiguousarray(m[rows].T.astype(ml_dtypes.bfloat16)),
                "w2T": w2t,
            }
        )
    return in_maps


def gather(results):
    # results: list of dicts with "out" [MP, NSH] per core -> [BSZ, SL, MP]
    cols = np.concatenate([results[c]["out"] for c in range(NCORES)], axis=1)
    return np.ascontiguousarray(cols.T).reshape(BSZ, SL, MP)


def kernel(repres, max_att, weight, **kw):
    from concourse.bass_utils import run_bass_kernel_spmd

    nc = get_nc()
    in_maps = make_in_maps(repres, max_att, weight)
    res = run_bass_kernel_spmd(nc, in_maps, list(range(NCORES)))
    return gather(res.results)

